# revision 1
# baseline (speedup 1.0000x reference)
"""Causal self-attention Trainium2 kernel (8-core head-parallel tensor parallel).

Strategy:
  - 16 heads split across 8 cores (2 heads each).
  - Host prep: x^T (shared), per-core W_qkv slice (transposed), per-core
    W_proj column-slice (transposed), per-core qkv bias slice.
  - Device (per core, all in a transposed "feature-major" dataflow):
      qkv^T = W_slice^T.T @ x^T   (+bias, via ACT eviction)   [384, B*T]
      per (batch b, head hh, q-chunk of 512):
        S^T[k,q]   = K^T.T @ Q^T            (PE, f32r)
        expS       = exp(0.125 * (S^T + causal_mask))  (DVE mask add on
                     diagonal tiles only, ACT exp eviction)
        [y^T; Z]   = [V | 1]^T.T @ expS     (PE accumulate over k-tiles;
                     row 64 = sum(exp) for free)
        y^T       /= Z                      (DVE mul with DMA-broadcast 1/Z)
      out^T partial = Wp_slice^T.T @ y^T    (PE)  -> DRAM [C, B*T]
  - Host: sum 8 partial out^T, transpose, +b_proj.
"""

import sys

if "/opt/trn_rl_repo" not in sys.path:
    sys.path.insert(0, "/opt/trn_rl_repo")

import numpy as np

# ---- problem constants (hardcoded for the grading harness) ----
B, T, C, H = 2, 2048, 1024, 16
HD = C // H            # 64
N_CORES = 8
HPC = H // N_CORES     # heads per core = 2

# dataflow dtype knobs
_F32R = True           # use float32r fast-path matmuls


def _cfg_full():
    return dict(B=B, T=T, C=C, HPC=HPC, f32r=_F32R)


def build_nc(cfg):
    """Build the single-core SPMD Bass program."""
    import concourse.bacc as bacc
    import concourse.mybir as mybir
    import concourse.tile as tile
    from concourse.masks import make_identity

    Bc, Tc, Cc, hpc = cfg["B"], cfg["T"], cfg["C"], cfg["HPC"]
    f32r = mybir.dt.float32r if cfg["f32r"] else mybir.dt.float32
    f32 = mybir.dt.float32
    bf16 = mybir.dt.bfloat16
    BT = Bc * Tc
    MQ = hpc * HD                 # rows per m-group (q|k|v) = 128
    assert MQ == 128
    KT_C = Cc // 128              # contraction tiles for qkv/x
    TOKC = 512
    NCH = BT // TOKC              # token chunks over both batches
    QC = Tc // TOKC               # q-chunks per batch
    KTT = Tc // 128               # k-tiles per batch
    MO = Cc // 128                # proj output tiles
    CH_PER_B = Tc // TOKC         # chunks per batch

    nc = bacc.Bacc()
    xT = nc.declare_dram_parameter("xT", [Cc, BT], f32r, isOutput=False)
    wqkvT = nc.declare_dram_parameter("wqkvT", [Cc, 3 * MQ], f32r, isOutput=False)
    bqkv = nc.declare_dram_parameter("bqkv", [3 * MQ, 1], f32, isOutput=False)
    wpT = nc.declare_dram_parameter("wpT", [MQ, Cc], bf16, isOutput=False)
    outT = nc.declare_dram_parameter("outT", [Cc, BT], f32, isOutput=True)

    xT_r = xT.rearrange("(kt p) t -> p kt t", p=128)
    wq_r = wqkvT.rearrange("(kt p) m -> p kt m", p=128)
    bq_r = bqkv.rearrange("(g p) o -> p (g o)", p=128)

    AF = mybir.ActivationFunctionType

    with tile.TileContext(nc) as tc:
        with (
            tc.tile_pool(name="consts", bufs=1) as consts,
            tc.tile_pool(name="xpool", bufs=3) as xpool,
            tc.tile_pool(name="spool", bufs=6) as spool,
            tc.tile_pool(name="tpool", bufs=3) as tpool,
            tc.tile_pool(name="ypool", bufs=3) as ypool,
            tc.tile_pool(name="npool", bufs=3) as npool,
            tc.tile_pool(name="opool", bufs=4) as opool,
            tc.tile_pool(name="ps_mm", bufs=4, space="PSUM") as ps_mm,
            tc.tile_pool(name="ps_y", bufs=2, space="PSUM") as ps_y,
            tc.tile_pool(name="ps_aux", bufs=2, space="PSUM") as ps_aux,
        ):
            # ---- constants ----
            w_sb = consts.tile([128, KT_C, 3 * MQ], f32r, tag="w")
            nc.sync.dma_start(out=w_sb, in_=wq_r)
            b_sb = consts.tile([128, 3], f32, tag="b")
            nc.sync.dma_start(out=b_sb, in_=bq_r)
            wp_sb = consts.tile([128, Cc], bf16, tag="wp")
            nc.sync.dma_start(out=wp_sb, in_=wpT[:, :])
            ident = consts.tile([128, 128], f32, tag="ident")
            make_identity(nc, ident)
            # 4 causal mask tiles (additive, 0 keep / -1e30 drop), offset o =
            # k0-q0 in {0,128,256,384}: keep iff q >= k iff f - p - o >= 0.
            masks = consts.tile([128, 4, TOKC], f32, tag="masks")
            for oi in range(4):
                m = masks[:, oi, :]
                nc.gpsimd.memset(m, 0.0)
                nc.gpsimd.affine_select(
                    out=m, in_=m,
                    compare_op=mybir.AluOpType.is_ge,
                    fill=-1e30,
                    base=-(oi * 128),
                    pattern=[[1, TOKC]],
                    channel_multiplier=-1,
                )

            # qkv^T buffers: q and k feature-major [128, BT]
            ones_f32 = consts.tile([128, HD], f32, tag="ones_f")
            nc.vector.memset(ones_f32[:, :], 1.0)
            ones_sb = consts.tile([1, HD], f32r, tag="ones")
            ident_bf = consts.tile([128, 128], bf16, tag="ident_bf")
            nc.vector.tensor_copy(ident_bf[:, :], ident[:, :])
            nc.scalar.activation(out=ones_sb[:, :], in_=ones_f32[0:1, :],
                                 func=AF.Copy)
            qT_sb = consts.tile([128, BT], f32r, tag="qT")
            kT_sb = consts.tile([128, BT], f32r, tag="kT")
            vT_sb = consts.tile([128, BT], bf16, tag="vT")

            # ---- phase 1: QKV projection (feature-major) ----
            for ch in range(NCH):
                x_t = xpool.tile([128, KT_C, TOKC], f32r, tag="x")
                nc.sync.dma_start(out=x_t, in_=xT_r[:, :, ch * TOKC:(ch + 1) * TOKC])
                b_idx = ch // CH_PER_B
                bcol = (ch % CH_PER_B) * TOKC
                for m in range(3):
                    ps = ps_mm.tile([128, TOKC], f32, tag="mm")
                    for kt in range(KT_C):
                        nc.tensor.matmul(
                            ps[:, :],
                            w_sb[:, kt, m * MQ:(m + 1) * MQ],
                            x_t[:, kt, :],
                            start=(kt == 0), stop=(kt == KT_C - 1),
                        )
                    dst = (qT_sb, kT_sb, vT_sb)[m]
                    nc.scalar.activation(
                        out=dst[:, ch * TOKC:(ch + 1) * TOKC], in_=ps[:, :],
                        func=AF.Identity, bias=b_sb[:, m:m + 1], scale=1.0,
                    )

            # ---- phase 2: V transpose -> per b: [128, kt, 2*65] f32r ----
            # cols [hh*65 : hh*65+64] = V rows of head hh, col hh*65+64 = 1.0
            v_sb = [
                consts.tile([128, KTT, 2 * 65], bf16, tag=f"v{b}",
                            name=f"v{b}") for b in range(Bc)
            ]
            for b in range(Bc):
                for kt in range(KTT):
                    for hh in range(hpc):
                        nc.scalar.activation(
                            out=v_sb[b][:, kt, hh * 65 + 64:hh * 65 + 65],
                            in_=ones_f32[:, 0:1], func=AF.Copy,
                        )
                    ps_t = ps_aux.tile([128, 128], bf16, tag="aux")
                    nc.tensor.transpose(
                        ps_t[:, :],
                        vT_sb[:, b * Tc + kt * 128:b * Tc + (kt + 1) * 128],
                        ident_bf[:, :],
                    )
                    for hh in range(hpc):
                        nc.scalar.activation(
                            out=v_sb[b][:, kt, hh * 65:hh * 65 + 64],
                            in_=ps_t[:, hh * HD:(hh + 1) * HD],
                            func=AF.Copy,
                        )

            # ---- phase 3: attention + phase 4: projection, per (b, q-chunk) ----
            for b in range(Bc):
                for qc in range(QC):
                    yT_t = ypool.tile([128, TOKC], bf16, tag="yT")
                    q_sl = slice(b * Tc + qc * TOKC, b * Tc + (qc + 1) * TOKC)
                    for hh in range(hpc):
                        n_kt = (qc + 1) * (TOKC // 128)
                        psy = ps_y.tile([65, TOKC], f32, tag="y")
                        # software-pipelined S / AV emission
                        exp_tiles = {}

                        def emit_S(kt):
                            pss = ps_mm.tile([128, TOKC], f32, tag="mm")
                            nc.tensor.matmul(
                                pss[:, :],
                                kT_sb[hh * HD:(hh + 1) * HD,
                                      b * Tc + kt * 128:b * Tc + (kt + 1) * 128],
                                qT_sb[hh * HD:(hh + 1) * HD, q_sl],
                                start=True, stop=True,
                            )
                            e_t = spool.tile([128, TOKC], bf16, tag="e")
                            di = kt - qc * (TOKC // 128)
                            if di >= 0:  # diagonal tile: add causal mask first
                                tmp = tpool.tile([128, TOKC], f32, tag="tmp")
                                nc.vector.tensor_add(tmp[:, :], pss[:, :],
                                                     masks[:, di, :])
                                src = tmp
                            else:
                                src = pss
                            nc.scalar.activation(out=e_t[:, :], in_=src[:, :],
                                                 func=AF.Exp, scale=0.125)
                            exp_tiles[kt] = e_t

                        def emit_AV(kt):
                            nc.tensor.matmul(
                                psy[:, :],
                                v_sb[b][:, kt, hh * 65:(hh + 1) * 65],
                                exp_tiles.pop(kt)[:, :],
                                start=(kt == 0), stop=(kt == n_kt - 1),
                            )

                        DEPTH = 3
                        for kt in range(n_kt):
                            emit_S(kt)
                            if kt >= DEPTH:
                                emit_AV(kt - DEPTH)
                        for kt in range(max(0, n_kt - DEPTH), n_kt):
                            emit_AV(kt)

                        # normalize: y^T[:, q] /= Z[q]  (PE rank-1 broadcast)
                        rc = npool.tile([1, TOKC], f32r, tag="rc")
                        with nc.allow_low_precision(reason="1/Z in f32r feeds PE broadcast"):
                            nc.vector.reciprocal(rc[:, :], psy[64:65, :])
                        ps_bc = ps_aux.tile([HD, TOKC], f32, tag="aux")
                        nc.tensor.matmul(ps_bc[:, :], ones_sb[:, :], rc[:, :],
                                         start=True, stop=True)
                        rc_bc = npool.tile([HD, TOKC], f32, tag="rcb")
                        nc.scalar.activation(out=rc_bc[:, :], in_=ps_bc[:, :],
                                             func=AF.Copy)
                        nc.vector.tensor_mul(
                            yT_t[hh * HD:(hh + 1) * HD, :],
                            psy[0:HD, :], rc_bc[:, :],
                        )

                    # projection for this (b, q-chunk)
                    for mo in range(MO):
                        pso = ps_mm.tile([128, TOKC], f32, tag="mm")
                        nc.tensor.matmul(
                            pso[:, :],
                            wp_sb[:, mo * 128:(mo + 1) * 128],
                            yT_t[:, :],
                            start=True, stop=True,
                        )
                        o_t = opool.tile([128, TOKC], f32, tag="o")
                        nc.vector.tensor_copy(o_t[:, :], pso[:, :])
                        nc.sync.dma_start(
                            out=outT[mo * 128:(mo + 1) * 128, q_sl],
                            in_=o_t[:, :],
                        )

    nc.finalize()
    return nc


def prep_inputs(cfg, x, W_attn, b_attn, W_proj, b_proj):
    """Host-side sharding: returns per-core input dicts."""
    Bc, Tc, Cc, hpc = cfg["B"], cfg["T"], cfg["C"], cfg["HPC"]
    n_cores = (Cc // HD) // hpc
    BT = Bc * Tc
    MQ = hpc * HD

    x = np.ascontiguousarray(x, dtype=np.float32)
    xT = np.ascontiguousarray(x.reshape(BT, Cc).T)

    in_maps = []
    for c in range(n_cores):
        r0 = c * MQ
        rows = []
        for g in range(3):
            rows.append(np.arange(g * Cc + r0, g * Cc + r0 + MQ))
        rows = np.concatenate(rows)
        w_slice = W_attn[rows, :]                       # [384, C]
        wqkvT = np.ascontiguousarray(w_slice.T)         # [C, 384]
        bq = np.ascontiguousarray(b_attn[rows].reshape(MQ * 3, 1))
        import ml_dtypes
        wpT = np.ascontiguousarray(W_proj[:, r0:r0 + MQ].T).astype(ml_dtypes.bfloat16)
        in_maps.append({
            "xT": xT,
            "wqkvT": wqkvT.astype(np.float32),
            "bqkv": bq.astype(np.float32),
            "wpT": wpT,
        })
    return in_maps


def combine(cfg, results, b_proj):
    Bc, Tc, Cc = cfg["B"], cfg["T"], cfg["C"]
    acc = results[0]["outT"].astype(np.float32).copy()
    for r in results[1:]:
        acc += r["outT"]
    out = acc.T + b_proj[None, :]
    return np.ascontiguousarray(out.reshape(Bc, Tc, Cc).astype(np.float32))


_NC_CACHE = {}


def kernel(x, W_attn, b_attn, W_proj, b_proj):
    from concourse.bass_utils import run_bass_kernel_spmd

    cfg = _cfg_full()
    key = "full"
    if key not in _NC_CACHE:
        _NC_CACHE[key] = build_nc(cfg)
    nc = _NC_CACHE[key]
    in_maps = prep_inputs(cfg, np.asarray(x), np.asarray(W_attn),
                          np.asarray(b_attn), np.asarray(W_proj),
                          np.asarray(b_proj))
    res = run_bass_kernel_spmd(nc, in_maps, list(range(N_CORES)))
    return combine(cfg, res.results, np.asarray(b_proj, dtype=np.float32))



# revision 10
# speedup vs baseline: 1.6918x; 1.6918x over previous
"""Causal self-attention Trainium2 kernel (8-core head-parallel tensor parallel).

v2 strategy (bf16 dataflow, ACT/PE balanced):
  - 16 heads split across 8 cores (2 heads each).
  - Host prep: x^T bf16 (shared), per-core W_qkv slice^T bf16, per-core
    W_proj column-slice^T bf16, per-core qkv bias slice f32.
  - Device per core, feature-major dataflow, emission interleaved as
    chunk(ch) -> attention(b,qc) -> chunk(ch+1) -> proj(b,qc) -> ...:
      qkv^T = W^T.T @ x^T (+bias via DVE tensor_scalar_add)  [384, BT] bf16
      V transposed per 128-token tile into [k, (V_h0|ones|V_h1)] layout
      per (b, q-chunk):
        per k-tile kt (both heads), diag-trimmed to the causal region:
          S^T[k,q] = K^T.T @ Q^T     (PE bf16, psum [128,2,512] pair)
          expS     = exp(0.125*S^T)  (one ACT op over both heads' banks)
          diag boundary band zeroed post-exp (GpSimd affine_select)
          psy[65,2,512] += [V|1]^T.T @ expS  (PE, row with ones = Z)
        Z -> sbuf (ACT), broadcast 1/.. via one PE matmul (E2), wide DVE
        reciprocal [128,512], 2 DVE muls -> y^T bf16 [128, 512]
      out^T partial = Wp^T.T @ y^T  (PE) -> DVE copy bf16 -> DRAM
  - Host: sum 8 bf16 partials in f32, transpose, +b_proj.
"""

import sys

if "/opt/trn_rl_repo" not in sys.path:
    sys.path.insert(0, "/opt/trn_rl_repo")

import numpy as np

# ---- problem constants (hardcoded for the grading harness) ----
B, T, C, H = 2, 2048, 1024, 16
HD = C // H            # 64
N_CORES = 8
HPC = H // N_CORES     # heads per core = 2


def _cfg_full():
    return dict(B=B, T=T, C=C, HPC=HPC)


def build_nc(cfg):
    """Build the single-core SPMD Bass program."""
    import concourse.bacc as bacc
    import concourse.mybir as mybir
    import concourse.tile as tile
    from concourse.masks import make_identity

    Bc, Tc, Cc, hpc = cfg["B"], cfg["T"], cfg["C"], cfg["HPC"]
    f32 = mybir.dt.float32
    bf16 = mybir.dt.bfloat16
    BT = Bc * Tc
    MQ = hpc * HD                 # rows per m-group (q|k|v) = 128
    assert MQ == 128
    KT_C = Cc // 128              # contraction tiles for qkv/x = 8
    TOKC = 512
    NCH = BT // TOKC              # token chunks over both batches = 8
    QC = Tc // TOKC               # q-chunks per batch = 4
    KTT = Tc // 128               # k-tiles per batch = 16
    MO = Cc // 128                # proj output tiles = 8
    CH_PER_B = Tc // TOKC         # chunks per batch = 4

    nc = bacc.Bacc()
    xT = nc.declare_dram_parameter("xT", [Cc, BT], bf16, isOutput=False)
    wqkvT = nc.declare_dram_parameter("wqkvT", [Cc, 3 * MQ], bf16, isOutput=False)
    bqkv = nc.declare_dram_parameter("bqkv", [3 * MQ, 1], f32, isOutput=False)
    wpT = nc.declare_dram_parameter("wpT", [MQ, Cc], bf16, isOutput=False)
    outT = nc.declare_dram_parameter("outT", [Cc, BT], bf16, isOutput=True)

    xT_r = xT.rearrange("(kt p) t -> p kt t", p=128)
    wq_r = wqkvT.rearrange("(kt p) m -> p kt m", p=128)
    bq_r = bqkv.rearrange("(g p) o -> p (g o)", p=128)

    AF = mybir.ActivationFunctionType

    with tile.TileContext(nc) as tc:
        with (
            tc.tile_pool(name="consts", bufs=1) as consts,
            tc.tile_pool(name="xpool", bufs=3) as xpool,
            tc.tile_pool(name="epool", bufs=5) as epool,
            tc.tile_pool(name="ypool", bufs=2) as ypool,
            tc.tile_pool(name="zpool", bufs=2) as zpool,
            tc.tile_pool(name="rpool", bufs=2) as rpool,
            tc.tile_pool(name="opool", bufs=4) as opool,
            tc.tile_pool(name="ps_s", bufs=2, space="PSUM") as ps_s,
            tc.tile_pool(name="ps_y", bufs=1, space="PSUM") as ps_y,
            tc.tile_pool(name="ps_misc", bufs=2, space="PSUM") as ps_misc,
        ):
            # ---- constants ----
            w_sb = consts.tile([128, KT_C, 3 * MQ], bf16, tag="w")
            nc.sync.dma_start(out=w_sb, in_=wq_r)
            b_sb = consts.tile([128, 3], f32, tag="b")
            nc.sync.dma_start(out=b_sb, in_=bq_r)
            # wp is first needed by the proj of block 0, well after startup;
            # load it behind the first x chunk (separate queue position).
            wp_sb = consts.tile([128, Cc], bf16, tag="wp")
            nc.scalar.dma_start(out=wp_sb, in_=wpT[:, :])
            ident = consts.tile([128, 128], f32, tag="ident")
            make_identity(nc, ident)
            ident_bf = consts.tile([128, 128], bf16, tag="ident_bf")
            nc.vector.tensor_copy(ident_bf[:, :], ident[:, :])
            # ones row vector for Z broadcast (rank-1 matmul)
            ones1 = consts.tile([1, HD], bf16, tag="ones1")
            nc.vector.memset(ones1[:, :], 1.0)

            # qkv feature-major buffers [128, BT] bf16
            qT_sb = consts.tile([128, BT], bf16, tag="qT")
            kT_sb = consts.tile([128, BT], bf16, tag="kT")
            vT_sb = consts.tile([128, BT], bf16, tag="vT")
            # transposed V per batch: [128 tok, kt, (V_h0|ones|V_h1|ones)]
            v_sb = [
                consts.tile([128, KTT, 2 * (HD + 1)], bf16, tag=f"v{b}",
                            name=f"v{b}") for b in range(Bc)
            ]
            for b in range(Bc):
                nc.vector.memset(v_sb[b][:, :, HD:HD + 1], 1.0)
                nc.vector.memset(v_sb[b][:, :, 2 * HD + 1:2 * HD + 2], 1.0)

            # ---- emission helpers ----
            def emit_chunk(ch):
                """QKV projection + V transpose for one 512-token chunk."""
                x_t = xpool.tile([128, KT_C, TOKC], bf16, tag="x")
                nc.sync.dma_start(out=x_t, in_=xT_r[:, :, ch * TOKC:(ch + 1) * TOKC])
                b = ch // CH_PER_B
                # v first: its eviction (DVE) finishes during the q/k matmuls
                # so the V transposes below don't stall the PE.
                for m in (2, 0, 1):
                    ps = ps_misc.tile([128, TOKC], f32, tag="mm")
                    for kt in range(KT_C):
                        nc.tensor.matmul(
                            ps[:, :],
                            w_sb[:, kt, m * MQ:(m + 1) * MQ],
                            x_t[:, kt, :],
                            start=(kt == 0), stop=(kt == KT_C - 1),
                        )
                    dst = (qT_sb, kT_sb, vT_sb)[m]
                    nc.vector.tensor_scalar_add(
                        dst[:, ch * TOKC:(ch + 1) * TOKC], ps[:, :],
                        b_sb[:, m:m + 1],
                    )
                # V transpose for this chunk's 4 k-tiles
                kt0 = (ch % CH_PER_B) * (TOKC // 128)
                for j in range(TOKC // 128):
                    tok = ch * TOKC + j * 128
                    ps_t = ps_misc.tile([128, 128], bf16, tag="mm")
                    nc.tensor.transpose(
                        ps_t[:, :], vT_sb[:, tok:tok + 128], ident_bf[:, :],
                    )
                    nc.vector.tensor_copy(
                        v_sb[b][:, kt0 + j, 0:HD], ps_t[:, 0:HD])
                    nc.vector.tensor_copy(
                        v_sb[b][:, kt0 + j, HD + 1:2 * HD + 1],
                        ps_t[:, HD:2 * HD])
                    # layout per kt: [V_h0(0:64) | ones(64) | V_h1(65:129) | ones(129)]

            def emit_attention(b, qc):
                """Attention for one (batch, q-chunk); returns yT bf16 tile."""
                n_kt = (qc + 1) * (TOKC // 128)
                q0 = b * Tc + qc * TOKC
                psy = ps_y.tile([128, 2, TOKC], f32, tag="y")
                pend = []   # (kt, lo, e2)

                def emit_av(kt, lo, e2):
                    for hh in range(hpc):
                        nc.tensor.matmul(
                            psy[0:HD + 1, hh, lo:TOKC],
                            v_sb[b][:, kt,
                                    (HD + 1) * hh:(HD + 1) * hh + HD + 1],
                            e2[:, hh, lo:TOKC],
                            start=(kt == 0), stop=(kt == n_kt - 1),
                        )

                for kt in range(n_kt):
                    di = kt - qc * (TOKC // 128)
                    lo = 128 * di if di >= 0 else 0
                    ps2 = ps_s.tile([128, 2, TOKC], f32, tag="s")
                    for hh in range(hpc):
                        nc.tensor.matmul(
                            ps2[:, hh, lo:TOKC],
                            kT_sb[HD * hh:HD * (hh + 1),
                                  b * Tc + kt * 128:b * Tc + (kt + 1) * 128],
                            qT_sb[HD * hh:HD * (hh + 1), q0 + lo:q0 + TOKC],
                            start=True, stop=True,
                        )
                    e2 = epool.tile([128, 2, TOKC], bf16, tag="e")
                    nc.scalar.activation(
                        out=e2[:, :, lo:TOKC], in_=ps2[:, :, lo:TOKC],
                        func=AF.Exp, scale=0.125,
                    )
                    if di >= 0:
                        # zero the upper-triangular part of the boundary band
                        nc.gpsimd.affine_select(
                            out=e2[:, :, lo:lo + 128], in_=e2[:, :, lo:lo + 128],
                            compare_op=mybir.AluOpType.is_ge,
                            fill=0.0, base=0,
                            pattern=[[0, 2], [1, 128]],
                            channel_multiplier=-1,
                        )
                    pend.append((kt, lo, e2))
                    if len(pend) > 1:
                        emit_av(*pend.pop(0))
                emit_av(*pend.pop(0))

                # normalize: yT = psy_y / Z  (Z on row 64 for both heads)
                z_h = []
                for hh in range(hpc):
                    z = zpool.tile([1, TOKC], bf16, tag="z", name=f"z{hh}")
                    nc.scalar.activation(out=z[:, :], in_=psy[HD:HD + 1, hh, :],
                                         func=AF.Copy)
                    z_h.append(z)
                ps_bc = ps_misc.tile([128, TOKC], f32, tag="mm")
                for hh in range(hpc):
                    nc.tensor.matmul(ps_bc[HD * hh:HD * (hh + 1), :],
                                     ones1[:, :], z_h[hh][:, :],
                                     start=True, stop=True)
                rc = rpool.tile([128, TOKC], f32, tag="rc")
                nc.vector.reciprocal(rc[:, :], ps_bc[:, :])
                yT = ypool.tile([128, TOKC], bf16, tag="yT")
                for hh in range(hpc):
                    nc.vector.tensor_mul(yT[HD * hh:HD * (hh + 1), :],
                                         psy[0:HD, hh, :],
                                         rc[HD * hh:HD * (hh + 1), :])
                return yT

            def emit_proj(b, qc, yT):
                q_sl = slice(b * Tc + qc * TOKC, b * Tc + (qc + 1) * TOKC)
                for mo in range(MO):
                    pso = ps_misc.tile([128, TOKC], f32, tag="mm")
                    nc.tensor.matmul(
                        pso[:, :], wp_sb[:, mo * 128:(mo + 1) * 128], yT[:, :],
                        start=True, stop=True,
                    )
                    o_t = opool.tile([128, TOKC], bf16, tag="o")
                    nc.vector.tensor_copy(o_t[:, :], pso[:, :])
                    nc.sync.dma_start(
                        out=outT[mo * 128:(mo + 1) * 128, q_sl], in_=o_t[:, :],
                    )

            # ---- main interleaved schedule ----
            pending = None
            for ch in range(NCH):
                emit_chunk(ch)
                if pending is not None:
                    emit_proj(*pending)
                b, qc = ch // CH_PER_B, ch % CH_PER_B
                yT = emit_attention(b, qc)
                pending = (b, qc, yT)
            emit_proj(*pending)

    nc.finalize()
    return nc


def prep_inputs(cfg, x, W_attn, b_attn, W_proj, b_proj):
    """Host-side sharding: returns per-core input dicts."""
    import ml_dtypes
    Bc, Tc, Cc, hpc = cfg["B"], cfg["T"], cfg["C"], cfg["HPC"]
    n_cores = (Cc // HD) // hpc
    BT = Bc * Tc
    MQ = hpc * HD

    x = np.ascontiguousarray(x, dtype=np.float32)
    xT = np.ascontiguousarray(x.reshape(BT, Cc).T).astype(ml_dtypes.bfloat16)

    in_maps = []
    for c in range(n_cores):
        r0 = c * MQ
        rows = []
        for g in range(3):
            rows.append(np.arange(g * Cc + r0, g * Cc + r0 + MQ))
        rows = np.concatenate(rows)
        w_slice = W_attn[rows, :]                       # [384, C]
        wqkvT = np.ascontiguousarray(w_slice.T).astype(ml_dtypes.bfloat16)
        bq = np.ascontiguousarray(b_attn[rows].reshape(MQ * 3, 1))
        wpT = np.ascontiguousarray(W_proj[:, r0:r0 + MQ].T).astype(ml_dtypes.bfloat16)
        in_maps.append({
            "xT": xT,
            "wqkvT": wqkvT,
            "bqkv": bq.astype(np.float32),
            "wpT": wpT,
        })
    return in_maps


def combine(cfg, results, b_proj):
    Bc, Tc, Cc = cfg["B"], cfg["T"], cfg["C"]
    acc = results[0]["outT"].astype(np.float32)
    for r in results[1:]:
        acc = acc + r["outT"].astype(np.float32)
    out = acc.T + b_proj[None, :]
    return np.ascontiguousarray(out.reshape(Bc, Tc, Cc).astype(np.float32))


_NC_CACHE = {}


def kernel(x, W_attn, b_attn, W_proj, b_proj):
    from concourse.bass_utils import run_bass_kernel_spmd

    cfg = _cfg_full()
    key = "full"
    if key not in _NC_CACHE:
        _NC_CACHE[key] = build_nc(cfg)
    nc = _NC_CACHE[key]
    in_maps = prep_inputs(cfg, np.asarray(x), np.asarray(W_attn),
                          np.asarray(b_attn), np.asarray(W_proj),
                          np.asarray(b_proj))
    res = run_bass_kernel_spmd(nc, in_maps, list(range(N_CORES)))
    return combine(cfg, res.results, np.asarray(b_proj, dtype=np.float32))


# revision 13
# speedup vs baseline: 1.7506x; 1.0348x over previous
"""Causal self-attention Trainium2 kernel (8-core head-parallel tensor parallel).

v2 strategy (bf16 dataflow, ACT/PE balanced):
  - 16 heads split across 8 cores (2 heads each).
  - Host prep: x^T bf16 (shared), per-core W_qkv slice^T bf16, per-core
    W_proj column-slice^T bf16, per-core qkv bias slice f32.
  - Device per core, feature-major dataflow, emission interleaved as
    chunk(ch) -> attention(b,qc) -> chunk(ch+1) -> proj(b,qc) -> ...:
      qkv^T = W^T.T @ x^T (+bias via DVE tensor_scalar_add)  [384, BT] bf16
      V transposed per 128-token tile into [k, (V_h0|ones|V_h1)] layout
      per (b, q-chunk):
        per k-tile kt (both heads), diag-trimmed to the causal region:
          S^T[k,q] = K^T.T @ Q^T     (PE bf16, psum [128,2,512] pair)
          expS     = exp(0.125*S^T)  (one ACT op over both heads' banks)
          diag boundary band zeroed post-exp (GpSimd affine_select)
          psy[65,2,512] += [V|1]^T.T @ expS  (PE, row with ones = Z)
        Z -> sbuf (ACT), broadcast 1/.. via one PE matmul (E2), wide DVE
        reciprocal [128,512], 2 DVE muls -> y^T bf16 [128, 512]
      out^T partial = Wp^T.T @ y^T  (PE) -> DVE copy bf16 -> DRAM
  - Host: sum 8 bf16 partials in f32, transpose, +b_proj.
"""

import sys

if "/opt/trn_rl_repo" not in sys.path:
    sys.path.insert(0, "/opt/trn_rl_repo")

import numpy as np

# ---- problem constants (hardcoded for the grading harness) ----
B, T, C, H = 2, 2048, 1024, 16
HD = C // H            # 64
N_CORES = 8
HPC = H // N_CORES     # heads per core = 2


def _cfg_full():
    return dict(B=B, T=T, C=C, HPC=HPC)


def build_nc(cfg):
    """Build the single-core SPMD Bass program."""
    import concourse.bacc as bacc
    import concourse.mybir as mybir
    import concourse.tile as tile
    from concourse.masks import make_identity

    Bc, Tc, Cc, hpc = cfg["B"], cfg["T"], cfg["C"], cfg["HPC"]
    f32 = mybir.dt.float32
    bf16 = mybir.dt.bfloat16
    BT = Bc * Tc
    MQ = hpc * HD                 # rows per m-group (q|k|v) = 128
    assert MQ == 128
    KT_C = Cc // 128              # contraction tiles for qkv/x = 8
    TOKC = 512
    NCH = BT // TOKC              # token chunks over both batches = 8
    QC = Tc // TOKC               # q-chunks per batch = 4
    KTT = Tc // 128               # k-tiles per batch = 16
    MO = Cc // 128                # proj output tiles = 8
    CH_PER_B = Tc // TOKC         # chunks per batch = 4

    nc = bacc.Bacc()
    xT = nc.declare_dram_parameter("xT", [Cc, BT], bf16, isOutput=False)
    wqkvT = nc.declare_dram_parameter("wqkvT", [Cc, 3 * MQ], bf16, isOutput=False)
    bqkv = nc.declare_dram_parameter("bqkv", [3 * MQ, 1], f32, isOutput=False)
    wpT = nc.declare_dram_parameter("wpT", [MQ, Cc], bf16, isOutput=False)
    outT = nc.declare_dram_parameter("outT", [Cc, BT], bf16, isOutput=True)

    xT_r = xT.rearrange("(kt p) t -> p kt t", p=128)
    wq_r = wqkvT.rearrange("(kt p) m -> p kt m", p=128)
    bq_r = bqkv.rearrange("(g p) o -> p (g o)", p=128)

    AF = mybir.ActivationFunctionType

    with tile.TileContext(nc) as tc:
        with (
            tc.tile_pool(name="consts", bufs=1) as consts,
            tc.tile_pool(name="xpool", bufs=3) as xpool,
            tc.tile_pool(name="epool", bufs=5) as epool,
            tc.tile_pool(name="ypool", bufs=2) as ypool,
            tc.tile_pool(name="zpool", bufs=2) as zpool,
            tc.tile_pool(name="rpool", bufs=2) as rpool,
            tc.tile_pool(name="opool", bufs=4) as opool,
            tc.tile_pool(name="ps_s", bufs=2, space="PSUM") as ps_s,
            tc.tile_pool(name="ps_y", bufs=1, space="PSUM") as ps_y,
            tc.tile_pool(name="ps_misc", bufs=2, space="PSUM") as ps_misc,
        ):
            # ---- constants ----
            w_sb = consts.tile([128, KT_C, 3 * MQ], bf16, tag="w")
            for kt in range(KT_C):
                nc.sync.dma_start(out=w_sb[:, kt, :], in_=wq_r[:, kt, :])
            b_sb = consts.tile([128, 3], f32, tag="b")
            nc.sync.dma_start(out=b_sb, in_=bq_r)
            # wp is first needed by the proj of block 0, well after startup;
            # load it behind the first x chunk (separate queue position).
            wp_sb = consts.tile([128, Cc], bf16, tag="wp")
            nc.scalar.dma_start(out=wp_sb, in_=wpT[:, :])
            ident = consts.tile([128, 128], f32, tag="ident")
            make_identity(nc, ident)
            ident_bf = consts.tile([128, 128], bf16, tag="ident_bf")
            nc.vector.tensor_copy(ident_bf[:, :], ident[:, :])
            # ones row vector for Z broadcast (rank-1 matmul)
            ones1 = consts.tile([1, HD], bf16, tag="ones1")
            nc.vector.memset(ones1[:, :], 1.0)

            # qkv feature-major buffers [128, BT] bf16
            qT_sb = consts.tile([128, BT], bf16, tag="qT")
            kT_sb = consts.tile([128, BT], bf16, tag="kT")
            vT_sb = consts.tile([128, BT], bf16, tag="vT")
            # transposed V per batch: [128 tok, kt, (V_h0|ones|V_h1|ones)]
            v_sb = [
                consts.tile([128, KTT, 2 * (HD + 1)], bf16, tag=f"v{b}",
                            name=f"v{b}") for b in range(Bc)
            ]
            for b in range(Bc):
                nc.vector.memset(v_sb[b][:, :, HD:HD + 1], 1.0)
                nc.vector.memset(v_sb[b][:, :, 2 * HD + 1:2 * HD + 2], 1.0)

            # ---- emission helpers ----
            def emit_chunk(ch):
                """QKV projection + V transpose for one 512-token chunk."""
                x_t = xpool.tile([128, KT_C, TOKC], bf16, tag="x")
                for kt in range(KT_C):
                    nc.sync.dma_start(
                        out=x_t[:, kt, :],
                        in_=xT_r[:, kt, ch * TOKC:(ch + 1) * TOKC])
                b = ch // CH_PER_B
                # v first: its eviction (DVE) finishes during the q/k matmuls
                # so the V transposes below don't stall the PE.
                for m in (2, 0, 1):
                    ps = ps_misc.tile([128, TOKC], f32, tag="mm")
                    for kt in range(KT_C):
                        nc.tensor.matmul(
                            ps[:, :],
                            w_sb[:, kt, m * MQ:(m + 1) * MQ],
                            x_t[:, kt, :],
                            start=(kt == 0), stop=(kt == KT_C - 1),
                        )
                    dst = (qT_sb, kT_sb, vT_sb)[m]
                    nc.vector.tensor_scalar_add(
                        dst[:, ch * TOKC:(ch + 1) * TOKC], ps[:, :],
                        b_sb[:, m:m + 1],
                    )
                # V transpose for this chunk's 4 k-tiles
                kt0 = (ch % CH_PER_B) * (TOKC // 128)
                for j in range(TOKC // 128):
                    tok = ch * TOKC + j * 128
                    ps_t = ps_misc.tile([128, 128], bf16, tag="mm")
                    nc.tensor.transpose(
                        ps_t[:, :], vT_sb[:, tok:tok + 128], ident_bf[:, :],
                    )
                    nc.vector.tensor_copy(
                        v_sb[b][:, kt0 + j, 0:HD], ps_t[:, 0:HD])
                    nc.vector.tensor_copy(
                        v_sb[b][:, kt0 + j, HD + 1:2 * HD + 1],
                        ps_t[:, HD:2 * HD])
                    # layout per kt: [V_h0(0:64) | ones(64) | V_h1(65:129) | ones(129)]

            def proj_steps(b, qc, yT):
                """Generator of single proj-tile emissions for one block."""
                q_sl = slice(b * Tc + qc * TOKC, b * Tc + (qc + 1) * TOKC)
                for mo in range(MO):
                    pso = ps_misc.tile([128, TOKC], f32, tag="mm")
                    nc.tensor.matmul(
                        pso[:, :], wp_sb[:, mo * 128:(mo + 1) * 128], yT[:, :],
                        start=True, stop=True,
                    )
                    o_t = opool.tile([128, TOKC], bf16, tag="o")
                    nc.vector.tensor_copy(o_t[:, :], pso[:, :])
                    nc.sync.dma_start(
                        out=outT[mo * 128:(mo + 1) * 128, q_sl], in_=o_t[:, :],
                    )
                    yield

            def emit_attention(b, qc, filler):
                """Attention for one (batch, q-chunk); interleaves `filler`
                (previous block's proj steps) into the kt pipeline so the PE
                stays fed while ACT works on exp. Returns yT bf16 tile."""
                n_kt = (qc + 1) * (TOKC // 128)
                q0 = b * Tc + qc * TOKC
                # spread the 8 proj tiles over the kt steps
                fill_per_step = -(-8 // n_kt) if filler is not None else 0
                psy = ps_y.tile([128, 2, TOKC], f32, tag="y")
                pend = []   # (kt, lo, e2)

                def emit_av(kt, lo, e2):
                    for hh in range(hpc):
                        nc.tensor.matmul(
                            psy[0:HD + 1, hh, lo:TOKC],
                            v_sb[b][:, kt,
                                    (HD + 1) * hh:(HD + 1) * hh + HD + 1],
                            e2[:, hh, lo:TOKC],
                            start=(kt == 0), stop=(kt == n_kt - 1),
                        )

                for kt in range(n_kt):
                    di = kt - qc * (TOKC // 128)
                    lo = 128 * di if di >= 0 else 0
                    ps2 = ps_s.tile([128, 2, TOKC], f32, tag="s")
                    for hh in range(hpc):
                        nc.tensor.matmul(
                            ps2[:, hh, lo:TOKC],
                            kT_sb[HD * hh:HD * (hh + 1),
                                  b * Tc + kt * 128:b * Tc + (kt + 1) * 128],
                            qT_sb[HD * hh:HD * (hh + 1), q0 + lo:q0 + TOKC],
                            start=True, stop=True,
                        )
                    e2 = epool.tile([128, 2, TOKC], bf16, tag="e")
                    nc.scalar.activation(
                        out=e2[:, :, lo:TOKC], in_=ps2[:, :, lo:TOKC],
                        func=AF.Exp, scale=0.125,
                    )
                    if di >= 0:
                        # zero the upper-triangular part of the boundary band
                        nc.gpsimd.affine_select(
                            out=e2[:, :, lo:lo + 128], in_=e2[:, :, lo:lo + 128],
                            compare_op=mybir.AluOpType.is_ge,
                            fill=0.0, base=0,
                            pattern=[[0, 2], [1, 128]],
                            channel_multiplier=-1,
                        )
                    if filler is not None:
                        for _ in range(fill_per_step):
                            if next(filler, "done") == "done":
                                filler = None
                                break
                    pend.append((kt, lo, e2))
                    if len(pend) > 1:
                        emit_av(*pend.pop(0))
                emit_av(*pend.pop(0))
                while filler is not None and next(filler, "done") != "done":
                    pass

                # normalize: yT = psy_y / Z  (Z on row 64 for both heads)
                z2 = zpool.tile([1, 2, TOKC], bf16, tag="z")
                nc.vector.tensor_copy(z2[:, :, :], psy[HD:HD + 1, :, :])
                ps_bc = ps_misc.tile([128, TOKC], f32, tag="mm")
                for hh in range(hpc):
                    nc.tensor.matmul(ps_bc[HD * hh:HD * (hh + 1), :],
                                     ones1[:, :], z2[:, hh, :],
                                     start=True, stop=True)
                rc = rpool.tile([128, TOKC], f32, tag="rc")
                nc.vector.reciprocal(rc[:, :], ps_bc[:, :])
                yT = ypool.tile([128, TOKC], bf16, tag="yT")
                for hh in range(hpc):
                    nc.vector.tensor_mul(yT[HD * hh:HD * (hh + 1), :],
                                         psy[0:HD, hh, :],
                                         rc[HD * hh:HD * (hh + 1), :])
                return yT

            # ---- main interleaved schedule ----
            pending = None
            for ch in range(NCH):
                emit_chunk(ch)
                b, qc = ch // CH_PER_B, ch % CH_PER_B
                yT = emit_attention(b, qc, pending)
                pending = proj_steps(b, qc, yT)
            for _ in pending:
                pass

    nc.finalize()
    return nc


def prep_inputs(cfg, x, W_attn, b_attn, W_proj, b_proj):
    """Host-side sharding: returns per-core input dicts."""
    import ml_dtypes
    Bc, Tc, Cc, hpc = cfg["B"], cfg["T"], cfg["C"], cfg["HPC"]
    n_cores = (Cc // HD) // hpc
    BT = Bc * Tc
    MQ = hpc * HD

    x = np.ascontiguousarray(x, dtype=np.float32)
    xT = np.ascontiguousarray(x.reshape(BT, Cc).T).astype(ml_dtypes.bfloat16)

    in_maps = []
    for c in range(n_cores):
        r0 = c * MQ
        rows = []
        for g in range(3):
            rows.append(np.arange(g * Cc + r0, g * Cc + r0 + MQ))
        rows = np.concatenate(rows)
        w_slice = W_attn[rows, :]                       # [384, C]
        wqkvT = np.ascontiguousarray(w_slice.T).astype(ml_dtypes.bfloat16)
        bq = np.ascontiguousarray(b_attn[rows].reshape(MQ * 3, 1))
        wpT = np.ascontiguousarray(W_proj[:, r0:r0 + MQ].T).astype(ml_dtypes.bfloat16)
        in_maps.append({
            "xT": xT,
            "wqkvT": wqkvT,
            "bqkv": bq.astype(np.float32),
            "wpT": wpT,
        })
    return in_maps


def combine(cfg, results, b_proj):
    Bc, Tc, Cc = cfg["B"], cfg["T"], cfg["C"]
    acc = results[0]["outT"].astype(np.float32)
    for r in results[1:]:
        acc = acc + r["outT"].astype(np.float32)
    out = acc.T + b_proj[None, :]
    return np.ascontiguousarray(out.reshape(Bc, Tc, Cc).astype(np.float32))


_NC_CACHE = {}


def kernel(x, W_attn, b_attn, W_proj, b_proj):
    from concourse.bass_utils import run_bass_kernel_spmd

    cfg = _cfg_full()
    key = "full"
    if key not in _NC_CACHE:
        _NC_CACHE[key] = build_nc(cfg)
    nc = _NC_CACHE[key]
    in_maps = prep_inputs(cfg, np.asarray(x), np.asarray(W_attn),
                          np.asarray(b_attn), np.asarray(W_proj),
                          np.asarray(b_proj))
    res = run_bass_kernel_spmd(nc, in_maps, list(range(N_CORES)))
    return combine(cfg, res.results, np.asarray(b_proj, dtype=np.float32))
